# revision 35
# baseline (speedup 1.0000x reference)
"""BinaryLinear Trainium2 kernel.

Computes y = x @ (sign(W) * scale[:, None]).T + bias for
x [131072, 256] f32, W [256, 256] f32, scale/bias [256] f32.

Data-parallel across 8 NeuronCores: each core takes a 16384-row shard of
x; W/scale/bias are replicated. Per core:

  prep (once): swT[ic] [128(i), 256(o)] = sign(W).T built on-device (ACT
  Sign + PE transposes). The binarized weights are exactly +/-1 in every
  matmul dtype, so they carry no rounding error. scale/bias are loaded as
  fp32 [128, 2, 256] partition-broadcast tiles for the epilogue.

  main loop (SB=4 row-tiles = 512 rows per DMA batch): a row permutation
  assigns each partition SB consecutive DRAM rows, so every partition's
  slice of a batched DMA is one contiguous 4KB segment (minimal
  descriptors; the output applies the same permutation). Per 128-row
  tile: 2 PE transposes (fp32 has no DMA transpose) -> one ACT copy
  evicts both [128,128] chunks from a shared PSUM bank into an SBUF tile
  in the matmul dtype -> 2 accumulating matmuls against swT. Two
  128-row tiles share one PSUM bank; a fp32 DVE mul (by scale) + add
  (bias) evicts them to SBUF, and the y batch DMAs out via gpsimd (so
  input and output use different DMA queue sets).

The matmul dtype is float32r: fp32 bits processed at full PE rate with
~11-bit mantissa rounding of the operands. Weights are exact (+/-1) and
scale/bias are applied in fp32, so the only error is the rounding of x
itself: measured 1.07e-4 max-rel vs the fp32 reference. The kernel is
HBM-bandwidth-bound: 33.6MB/core of mandatory fp32 I/O at ~360GB/s
gives a ~93us floor; measured ~110us mean / ~117us max across cores.
(A bit-exact variant — mm_dtype=float32 — measures ~178us, PE-bound on
the fp32 two-pass matmul.)
"""

from contextlib import ExitStack

import numpy as np

import concourse.bass as bass
import concourse.tile as tile
from concourse import bacc, mybir
from concourse import bass_utils
from concourse.masks import make_identity

F32 = mybir.dt.float32
AF = mybir.ActivationFunctionType

B_FULL = 131072
I_DIM = 256
O_DIM = 256
N_CORES = 8
P = 128


def build_kernel(b_rows: int, mm_dtype=F32):
    """Build + compile the per-core Bass program for a b_rows-row shard."""
    assert b_rows % P == 0
    ntiles = b_rows // P

    nc = bacc.Bacc("TRN2", target_bir_lowering=False, debug=False)
    x_d = nc.dram_tensor("x", [b_rows, I_DIM], F32, kind="ExternalInput").ap()
    w_d = nc.dram_tensor("w", [O_DIM, I_DIM], F32, kind="ExternalInput").ap()
    scale_d = nc.dram_tensor("scale", [O_DIM], F32, kind="ExternalInput").ap()
    bias_d = nc.dram_tensor("bias", [O_DIM], F32, kind="ExternalInput").ap()
    y_d = nc.dram_tensor("y", [b_rows, O_DIM], F32, kind="ExternalOutput").ap()

    with tile.TileContext(nc) as tc, ExitStack() as ctx:
        _emit(ctx, tc, y_d, x_d, w_d, scale_d, bias_d, ntiles, mm_dtype)

    nc.compile()
    return nc


def _emit(ctx, tc, y, x, w, scale, bias, ntiles, mm_dtype):
    nc = tc.nc

    singles = ctx.enter_context(tc.tile_pool(name="singles", bufs=1))
    xpool = ctx.enter_context(tc.tile_pool(name="xin", bufs=10))
    xtpool = ctx.enter_context(tc.tile_pool(name="xt", bufs=8))
    ypool = ctx.enter_context(tc.tile_pool(name="yout", bufs=8))
    psum_t = ctx.enter_context(tc.tile_pool(name="psum_t", bufs=4, space="PSUM"))
    psum_y = ctx.enter_context(tc.tile_pool(name="psum_y", bufs=4, space="PSUM"))

    ident = singles.tile([P, P], F32)
    make_identity(nc, ident)

    # ---- prep: swT[ic] = [128(i), 256(o)] with entries sign(W[o,i]) —
    # exactly ±1 in any matmul dtype, so the matmul weights carry no
    # rounding error. scale/bias are applied in fp32 at PSUM eviction.
    w_t = w.rearrange("(c p) i -> c p i", c=2)          # [2, 128, 256]
    swT = [singles.tile([P, O_DIM], mm_dtype, name=f"swT{ic}", tag=f"swT{ic}")
           for ic in range(2)]
    for oc in range(2):
        w_sb = singles.tile([P, I_DIM], F32, tag=f"w{oc}")
        nc.scalar.dma_start(out=w_sb, in_=w_t[oc])
        sg_sb = singles.tile([P, I_DIM], F32, tag=f"sg{oc}")
        nc.scalar.activation(sg_sb, w_sb, AF.Sign)
        for ic in range(2):
            pt = psum_t.tile([P, P], F32, tag="psum_tr")
            nc.tensor.transpose(pt, sg_sb[:, ic * P:(ic + 1) * P], ident)
            nc.vector.tensor_copy(out=swT[ic][:, oc * P:(oc + 1) * P], in_=pt)

    # scale broadcast across all 128 partitions (x2 in free dim), fp32
    scale_bc = singles.tile([P, 2, O_DIM], F32)
    scale_rep = bass.AP(tensor=scale.tensor, offset=scale.offset,
                        ap=[[0, P], [0, 2]] + list(scale.ap))
    nc.scalar.dma_start(out=scale_bc, in_=scale_rep)

    # bias broadcast across all 128 partitions (x2 in free dim), fp32
    bias_bc = singles.tile([P, 2, O_DIM], F32)
    bias_rep = bass.AP(tensor=bias.tensor, offset=bias.offset,
                       ap=[[0, P], [0, 2]] + list(bias.ap))
    nc.scalar.dma_start(out=bias_bc, in_=bias_rep)

    # ---- main loop: 4 row-tiles (512 rows) per DMA batch.
    # Row permutation: partition p holds SB *consecutive* DRAM rows, so each
    # partition's slice of a batched DMA is one contiguous SB*1KB segment
    # (fewer descriptors per DMA). The same permutation is applied on the
    # output side, so the result lands in the right place.
    SB = 2  # row-tiles per DMA batch
    assert ntiles % SB == 0
    x4 = x.rearrange("(n p s) i -> n p (s i)", p=P, s=SB)
    y4 = y.rearrange("(n p s) o -> n p (s o)", p=P, s=SB)
    for n in range(ntiles // SB):
        x_sb = xpool.tile([P, SB * I_DIM], F32, tag="x")
        nc.sync.dma_start(out=x_sb, in_=x4[n])

        y_sb = ypool.tile([P, SB // 2, 2, O_DIM], F32, tag="y")
        for sp in range(SB // 2):  # psum_y bank holds 2 row-tiles
            py = psum_y.tile([P, 2, O_DIM], F32, tag="py")
            for s2 in range(2):
                s = sp * 2 + s2
                pt = psum_t.tile([P, 2, P], F32, tag="psum_tr")
                for ic in range(2):
                    nc.tensor.transpose(
                        pt[:, ic],
                        x_sb[:, s * I_DIM + ic * P:s * I_DIM + (ic + 1) * P],
                        ident)
                xT = xtpool.tile([P, 2, P], mm_dtype, tag="xT")  # [i, chunk, b]
                nc.scalar.copy(out=xT, in_=pt)
                for ic in range(2):
                    nc.tensor.matmul(py[:, s2], lhsT=xT[:, ic], rhs=swT[ic],
                                     start=(ic == 0), stop=(ic == 1))
            # y = scale * (sum_i x_i * sign(w)) + bias, all fp32
            nc.vector.tensor_mul(out=y_sb[:, sp], in0=py, in1=scale_bc)
            nc.vector.tensor_add(out=y_sb[:, sp], in0=y_sb[:, sp], in1=bias_bc)
            nc.gpsimd.dma_start(
                out=y4[n][:, sp * 2 * O_DIM:(sp + 1) * 2 * O_DIM],
                in_=y_sb[:, sp])


_CACHE = {}


def _get_nc(b_rows, mm_dtype=F32):
    key = (b_rows, str(mm_dtype))
    if key not in _CACHE:
        _CACHE[key] = build_kernel(b_rows, mm_dtype)
    return _CACHE[key]


def run_sharded(x, W, scale, bias, trace=False, mm_dtype=F32):
    """Run the SPMD kernel on 8 cores; returns (y_full, BassKernelResults)."""
    x = np.ascontiguousarray(x, dtype=np.float32)
    W = np.ascontiguousarray(W, dtype=np.float32)
    scale = np.ascontiguousarray(scale, dtype=np.float32)
    bias = np.ascontiguousarray(bias, dtype=np.float32)
    b_shard = x.shape[0] // N_CORES
    nc = _get_nc(b_shard, mm_dtype)
    xs = x.reshape(N_CORES, b_shard, I_DIM)
    in_maps = [
        {"x": np.ascontiguousarray(xs[c]), "w": W, "scale": scale, "bias": bias}
        for c in range(N_CORES)
    ]
    def _run():
        return bass_utils.run_bass_kernel_spmd(
            nc, in_maps, core_ids=list(range(N_CORES)), trace=trace,
            trace_cores=list(range(N_CORES)) if trace else None,
        )

    try:
        res = _run()
    except Exception:  # one retry for transient device/runtime hiccups
        import time
        time.sleep(5)
        res = _run()
    y = np.concatenate([res.results[c]["y"] for c in range(N_CORES)], axis=0)
    return y, res


def kernel(x, W, scale, bias):
    y, _ = run_sharded(x, W, scale, bias, trace=False,
                       mm_dtype=mybir.dt.float32r)
    return y


# revision 36
# speedup vs baseline: 1.0394x; 1.0394x over previous
"""BinaryLinear Trainium2 kernel.

Computes y = x @ (sign(W) * scale[:, None]).T + bias for
x [131072, 256] f32, W [256, 256] f32, scale/bias [256] f32.

Data-parallel across 8 NeuronCores: each core takes a 16384-row shard of
x; W/scale/bias are replicated. Per core:

  prep (once): swT[ic] [128(i), 256(o)] = sign(W).T built on-device (ACT
  Sign + PE transposes). The binarized weights are exactly +/-1 in every
  matmul dtype, so they carry no rounding error. scale/bias are loaded as
  fp32 [128, 2, 256] partition-broadcast tiles for the epilogue.

  main loop (SB=4 row-tiles = 512 rows per DMA batch): a row permutation
  assigns each partition SB consecutive DRAM rows, so every partition's
  slice of a batched DMA is one contiguous 4KB segment (minimal
  descriptors; the output applies the same permutation). Per 128-row
  tile: 2 PE transposes (fp32 has no DMA transpose) -> one ACT copy
  evicts both [128,128] chunks from a shared PSUM bank into an SBUF tile
  in the matmul dtype -> 2 accumulating matmuls against swT. Two
  128-row tiles share one PSUM bank; a fp32 DVE mul (by scale) + add
  (bias) evicts them to SBUF, and the y batch DMAs out via gpsimd (so
  input and output use different DMA queue sets).

The matmul dtype is float32r: fp32 bits processed at full PE rate with
~11-bit mantissa rounding of the operands. Weights are exact (+/-1) and
scale/bias are applied in fp32, so the only error is the rounding of x
itself: measured 1.07e-4 max-rel vs the fp32 reference. The kernel is
HBM-bandwidth-bound: 33.6MB/core of mandatory fp32 I/O at ~360GB/s
gives a ~93us floor; measured ~110us mean / ~117us max across cores.
(A bit-exact variant — mm_dtype=float32 — measures ~178us, PE-bound on
the fp32 two-pass matmul.)
"""

from contextlib import ExitStack

import numpy as np

import concourse.bass as bass
import concourse.tile as tile
from concourse import bacc, mybir
from concourse import bass_utils
from concourse.masks import make_identity

F32 = mybir.dt.float32
AF = mybir.ActivationFunctionType

B_FULL = 131072
I_DIM = 256
O_DIM = 256
N_CORES = 8
P = 128


def build_kernel(b_rows: int, mm_dtype=F32):
    """Build + compile the per-core Bass program for a b_rows-row shard."""
    assert b_rows % P == 0
    ntiles = b_rows // P

    nc = bacc.Bacc("TRN2", target_bir_lowering=False, debug=False)
    x_d = nc.dram_tensor("x", [b_rows, I_DIM], F32, kind="ExternalInput").ap()
    w_d = nc.dram_tensor("w", [O_DIM, I_DIM], F32, kind="ExternalInput").ap()
    scale_d = nc.dram_tensor("scale", [O_DIM], F32, kind="ExternalInput").ap()
    bias_d = nc.dram_tensor("bias", [O_DIM], F32, kind="ExternalInput").ap()
    y_d = nc.dram_tensor("y", [b_rows, O_DIM], F32, kind="ExternalOutput").ap()

    with tile.TileContext(nc) as tc, ExitStack() as ctx:
        _emit(ctx, tc, y_d, x_d, w_d, scale_d, bias_d, ntiles, mm_dtype)

    nc.compile()
    return nc


def _emit(ctx, tc, y, x, w, scale, bias, ntiles, mm_dtype):
    nc = tc.nc

    singles = ctx.enter_context(tc.tile_pool(name="singles", bufs=1))
    xpool = ctx.enter_context(tc.tile_pool(name="xin", bufs=10))
    xtpool = ctx.enter_context(tc.tile_pool(name="xt", bufs=8))
    ypool = ctx.enter_context(tc.tile_pool(name="yout", bufs=8))
    psum_t = ctx.enter_context(tc.tile_pool(name="psum_t", bufs=4, space="PSUM"))
    psum_y = ctx.enter_context(tc.tile_pool(name="psum_y", bufs=4, space="PSUM"))

    ident = singles.tile([P, P], F32)
    make_identity(nc, ident)

    # ---- prep: swT[ic] = [128(i), 256(o)] with entries sign(W[o,i]) —
    # exactly ±1 in any matmul dtype, so the matmul weights carry no
    # rounding error. scale/bias are applied in fp32 at PSUM eviction.
    w_t = w.rearrange("(c p) i -> c p i", c=2)          # [2, 128, 256]
    swT = [singles.tile([P, O_DIM], mm_dtype, name=f"swT{ic}", tag=f"swT{ic}")
           for ic in range(2)]
    for oc in range(2):
        w_sb = singles.tile([P, I_DIM], F32, tag=f"w{oc}")
        nc.scalar.dma_start(out=w_sb, in_=w_t[oc])
        sg_sb = singles.tile([P, I_DIM], F32, tag=f"sg{oc}")
        nc.scalar.activation(sg_sb, w_sb, AF.Sign)
        for ic in range(2):
            pt = psum_t.tile([P, P], F32, tag="psum_tr")
            nc.tensor.transpose(pt, sg_sb[:, ic * P:(ic + 1) * P], ident)
            nc.vector.tensor_copy(out=swT[ic][:, oc * P:(oc + 1) * P], in_=pt)

    # scale broadcast across all 128 partitions (x2 in free dim), fp32
    scale_bc = singles.tile([P, 2, O_DIM], F32)
    scale_rep = bass.AP(tensor=scale.tensor, offset=scale.offset,
                        ap=[[0, P], [0, 2]] + list(scale.ap))
    nc.scalar.dma_start(out=scale_bc, in_=scale_rep)

    # bias broadcast across all 128 partitions (x2 in free dim), fp32
    bias_bc = singles.tile([P, 2, O_DIM], F32)
    bias_rep = bass.AP(tensor=bias.tensor, offset=bias.offset,
                       ap=[[0, P], [0, 2]] + list(bias.ap))
    nc.scalar.dma_start(out=bias_bc, in_=bias_rep)

    # ---- main loop: 4 row-tiles (512 rows) per DMA batch.
    # Row permutation: partition p holds SB *consecutive* DRAM rows, so each
    # partition's slice of a batched DMA is one contiguous SB*1KB segment
    # (fewer descriptors per DMA). The same permutation is applied on the
    # output side, so the result lands in the right place.
    SB = 4  # row-tiles per DMA batch
    assert ntiles % SB == 0
    x4 = x.rearrange("(n p s) i -> n p (s i)", p=P, s=SB)
    y4 = y.rearrange("(n p s) o -> n p (s o)", p=P, s=SB)
    for n in range(ntiles // SB):
        x_sb = xpool.tile([P, SB * I_DIM], F32, tag="x")
        nc.sync.dma_start(out=x_sb, in_=x4[n])

        y_sb = ypool.tile([P, SB // 2, 2, O_DIM], F32, tag="y")
        for sp in range(SB // 2):  # psum_y bank holds 2 row-tiles
            py = psum_y.tile([P, 2, O_DIM], F32, tag="py")
            for s2 in range(2):
                s = sp * 2 + s2
                pt = psum_t.tile([P, 2, P], F32, tag="psum_tr")
                for ic in range(2):
                    nc.tensor.transpose(
                        pt[:, ic],
                        x_sb[:, s * I_DIM + ic * P:s * I_DIM + (ic + 1) * P],
                        ident)
                xT = xtpool.tile([P, 2, P], mm_dtype, tag="xT")  # [i, chunk, b]
                nc.scalar.copy(out=xT, in_=pt)
                for ic in range(2):
                    nc.tensor.matmul(py[:, s2], lhsT=xT[:, ic], rhs=swT[ic],
                                     start=(ic == 0), stop=(ic == 1))
            # y = scale * (sum_i x_i * sign(w)) + bias, all fp32
            nc.vector.tensor_mul(out=y_sb[:, sp], in0=py, in1=scale_bc)
            nc.vector.tensor_add(out=y_sb[:, sp], in0=y_sb[:, sp], in1=bias_bc)
            nc.gpsimd.dma_start(
                out=y4[n][:, sp * 2 * O_DIM:(sp + 1) * 2 * O_DIM],
                in_=y_sb[:, sp])


_CACHE = {}


def _get_nc(b_rows, mm_dtype=F32):
    key = (b_rows, str(mm_dtype))
    if key not in _CACHE:
        _CACHE[key] = build_kernel(b_rows, mm_dtype)
    return _CACHE[key]


def run_sharded(x, W, scale, bias, trace=False, mm_dtype=F32):
    """Run the SPMD kernel on 8 cores; returns (y_full, BassKernelResults)."""
    x = np.ascontiguousarray(x, dtype=np.float32)
    W = np.ascontiguousarray(W, dtype=np.float32)
    scale = np.ascontiguousarray(scale, dtype=np.float32)
    bias = np.ascontiguousarray(bias, dtype=np.float32)
    b_shard = x.shape[0] // N_CORES
    nc = _get_nc(b_shard, mm_dtype)
    xs = x.reshape(N_CORES, b_shard, I_DIM)
    in_maps = [
        {"x": np.ascontiguousarray(xs[c]), "w": W, "scale": scale, "bias": bias}
        for c in range(N_CORES)
    ]
    def _run():
        return bass_utils.run_bass_kernel_spmd(
            nc, in_maps, core_ids=list(range(N_CORES)), trace=trace,
            trace_cores=list(range(N_CORES)) if trace else None,
        )

    try:
        res = _run()
    except Exception:  # one retry for transient device/runtime hiccups
        import time
        time.sleep(5)
        res = _run()
    y = np.concatenate([res.results[c]["y"] for c in range(N_CORES)], axis=0)
    return y, res


def kernel(x, W, scale, bias):
    y, _ = run_sharded(x, W, scale, bias, trace=False,
                       mm_dtype=mybir.dt.float32r)
    return y
